# revision 46
# baseline (speedup 1.0000x reference)
"""AttentionMixer kernel for 8 Trainium2 NeuronCores.

Sharding: data-parallel over (batch B=4) x (query-half NQ/2) -> 8 cores.
Each core computes, for its (b, half):
    q = meshT slice proj, k/v = pc proj (k/v work duplicated across the
    2 cores of a batch), masked softmax attention, Wo projection.
Layout is "transposed" throughout (features on partitions, tokens on the
free dim) so every matmul contracts over the partition dim natively:
    qT/kT: [e, n] via W.T as lhsT, xT as rhs (bf16)
    scoresT: [nk, nq] = kT_h.T-contract-d qT_h  (2 heads row-packed)
    attnT = exp(scoresT/8 + mask) in fp8e4m3, produced ALTERNATELY by
      - ScalarE: native Exp ACTIVATE (handles masked/partial blocks)
      - VectorE: Schraudolph bit-trick -- one tensor_scalar op computes
        int8(round(1.4427*s + 55.63)) whose int8 bits ARE the e4m3
        encoding of exp(s/8) to ~7% per-element (averages out over the
        ~2500-key softmax; tolerance is 2e-2).  Only fully-valid blocks
        (j < jfull on every core) go to VectorE so no mask is needed.
    ctxT_h: [65, nq] via fp8 DoubleRow matmuls -- one matmul contracts
      TWO 128-key blocks (virtual K=256), with a ones column appended to
      v giving the softmax denominator Z for free.  mix = ctx/Z @ Wo.T.
Projections/scores bf16 with fp32 PSUM accumulation; inputs pcT/meshT
are fp8 (error averages out through the 128-dim projections).

The j-loop is software-pipelined at block-pair granularity: scores+exp
of pair p are emitted before ctx of pair p-1 so PE never stalls behind
the current exp, and the two exp engines run concurrently.
"""

import math

import numpy as np
import ml_dtypes

import concourse.bass as bass
import concourse.bacc as bacc
import concourse.mybir as mybir
import concourse.tile as tile
from concourse.bass_utils import run_bass_kernel_spmd

B, NQ, NK, E, DPC, H = 4, 2048, 4096, 256, 128, 4
HD = E // H  # 64
NQH = NQ // 2  # per-core queries: 1024
NKB = NK // 128  # 32 nk blocks
P = 128
BF16 = mybir.dt.bfloat16
F32 = mybir.dt.float32
F8 = mybir.dt.float8e4
I8 = mybir.dt.int8
MASK_NEG = -80.0
# Schraudolph exp in e4m3-bit space: bits = round(SLOPE*s + CCONST)
# (DVE fp32->int8 convert is round-half-even, probed on HW).
SLOPE = 8 * math.log2(math.e) * 0.125   # 1.4427
CCONST = 55.63
VPAD = 80  # per-head stride in v_sb (65 used; *H must be 16B aligned)

_CACHE = {}


def build_nc(jmax=NKB, jfull=NKB):
    nc = bacc.Bacc(None)
    knt = (jmax + 3) // 4         # 512-wide kT tiles needed
    nch = (jmax + 7) // 8         # 1024-wide pcT DMA chunks needed
    npair = (jmax + 1) // 2       # ctx block-pairs (last may be single)
    odd = jmax % 2 == 1

    # blocks that must use ScalarE (need the mask bias); plus alternating
    # leading blocks so the two exp engines get ~equal work
    tail = list(range(jfull, jmax))
    n_act_extra = max(0, (jmax + 1) // 2 - len(tail))
    act_blocks = set(tail) | set(range(jfull)[1::2][:n_act_extra])

    # ---- DRAM params (per-core shapes; host stages exact SBUF layouts) ----
    meshT_d = nc.declare_dram_parameter("meshT", [P, 2, NQH], F8, False)
    pcT_d = nc.declare_dram_parameter("pcT", [P, NK], F8, False)
    wqT_d = nc.declare_dram_parameter("wqT", [P, 2, E], BF16, False)
    wkT_d = nc.declare_dram_parameter("wkT", [P, E], BF16, False)
    wvT_d = nc.declare_dram_parameter("wvT", [P, E], BF16, False)
    woT_d = nc.declare_dram_parameter("woT", [HD, H, E], BF16, False)
    # consts: [bk | bq | bop | maskb] along the free dim
    consts_d = nc.declare_dram_parameter("consts", [P, 6 + NKB], F32, False)
    mixT_d = nc.declare_dram_parameter("mixT", [2, P, NQH], BF16, isOutput=True)

    with tile.TileContext(nc) as tc:
        with (
            tc.tile_pool(name="const", bufs=1) as cpool,
            tc.tile_pool(name="acts", bufs=1) as apool,
            tc.tile_pool(name="attn", bufs=4) as attn_pool,
            tc.tile_pool(name="small", bufs=2) as spool,
            tc.tile_pool(name="ps_big", bufs=3, space="PSUM") as ps_big,
            tc.tile_pool(name="ps_ctx", bufs=2, space="PSUM") as ps_ctx,
        ):
            # ---- load constants / inputs into SBUF ----
            meshT = cpool.tile([P, 2, NQH], F8)
            pcT = cpool.tile([P, NK], F8)
            wqT = cpool.tile([P, 2, E], BF16)
            wkT = cpool.tile([P, E], BF16)
            wvT = cpool.tile([P, E], BF16)
            woT = cpool.tile([HD, H, E], BF16)
            consts = cpool.tile([P, 6 + NKB], F32)
            # consts[:, 0:2] holds bk, unused: the k-bias adds a per-query
            # constant to every key's score, which softmax cancels.
            bq = consts[:, 2:4]
            bop = consts[:, 4:6]
            maskb = consts[:, 6:6 + NKB]

            # critical-path inputs on the sync engine (HWDGE)
            nc.sync.dma_start(wkT[:], wkT_d[:, :])
            nc.sync.dma_start(consts[:], consts_d[:, :])
            nc.sync.dma_start(pcT[:, 0:512], pcT_d[:, 0:512])
            nc.sync.dma_start(wqT[:], wqT_d[:, :, :])
            nc.sync.dma_start(meshT[:, 0, :], meshT_d[:, 0, :])
            nc.sync.dma_start(meshT[:, 1, :], meshT_d[:, 1, :])
            nc.sync.dma_start(pcT[:, 512:1024], pcT_d[:, 512:1024])
            # bulk inputs ride the gpsimd SWDGE queue so they don't
            # serialize behind the critical-path loads above
            nc.gpsimd.dma_start(wvT[:], wvT_d[:, :])
            for ch in range(1, nch):
                nc.gpsimd.dma_start(pcT[:, ch * 1024:(ch + 1) * 1024],
                                    pcT_d[:, ch * 1024:(ch + 1) * 1024])
            nc.gpsimd.dma_start(woT[:], woT_d[:, :, :])

            # HAM warm-up: dependency-free matmuls during the input-DMA
            # window so the PE clock gate is at 2.4 GHz for the real work
            warm = cpool.tile([P, 512], BF16)
            nc.vector.memset(warm[:], 0.0)
            wps = ps_big.tile([P, 1024], F32, tag="big")
            for _ in range(20):
                nc.tensor.matmul(wps[:, 0:512], warm[:, 0:128], warm[:],
                                 start=True, stop=True)

            kT = apool.tile([P, 2, knt * 512], BF16)
            qT = apool.tile([P, 2, NQH], BF16)
            # v (fp8) in DoubleRow pair layout + ones column for Z
            v_sb = apool.tile([P, npair, 2, H * VPAD], F8)
            for h in range(H):
                nc.vector.memset(
                    v_sb[:, :, :, h * VPAD + HD:h * VPAD + HD + 1], 1.0)
            mixT = apool.tile([P, 2, NQH], BF16)
            ctxn = apool.tile([HD, H, NQH], BF16)  # normalized ctxT per head

            def k_proj_eb(eb, nt0, n_nt):
                # n_nt (1 or 2) 512-wide kT tiles for one e-block; the
                # PSUM->SBUF casts are split across ScalarE/VectorE so
                # neither exp engine eats the whole cost
                ps = ps_big.tile([P, 1024], F32, tag="big")
                for i in range(n_nt):
                    nc.tensor.matmul(
                        ps[:, i * 512:(i + 1) * 512],
                        wkT[:, eb * P:(eb + 1) * P],
                        pcT[:, (nt0 + i) * 512:(nt0 + i + 1) * 512],
                        start=True, stop=True,
                    )
                    # kT is pre-scaled by SLOPE so the DVE exp needs only a
                    # single ADD op and ACT exp just rescales
                    dst = kT[:, eb, (nt0 + i) * 512:(nt0 + i + 1) * 512]
                    if i == 0:
                        nc.scalar.activation(
                            dst, ps[:, 0:512],
                            mybir.ActivationFunctionType.Copy, scale=SLOPE)
                    else:
                        nc.vector.tensor_scalar_mul(
                            dst, ps[:, 512:1024], SLOPE)

            def q_proj_ebnt(eb, nt):
                ps = ps_big.tile([P, 1024], F32, tag="big")
                for cb in range(2):
                    nc.tensor.matmul(
                        ps[:, 0:512],
                        wqT[:, cb, eb * P:(eb + 1) * P],
                        meshT[:, cb, nt * 512:(nt + 1) * 512],
                        start=(cb == 0), stop=(cb == 1),
                    )
                nc.vector.tensor_scalar_add(
                    qT[:, eb, nt * 512:(nt + 1) * 512], ps[:, 0:512],
                    bq[:, eb:eb + 1])

            def v_pair(p, scalar_eng=False):
                # both blocks of a pair -> one ps tile, one engine copy
                # (gpsimd can't help: it has no PSUM access)
                blocks = [2 * p] if odd and p == npair - 1 else [2 * p, 2 * p + 1]
                ps = ps_big.tile([P, 1024], F32, tag="big")
                for ci, j in enumerate(blocks):
                    nc.tensor.matmul(
                        ps[:, ci * E:(ci + 1) * E],
                        pcT[:, j * P:(j + 1) * P],
                        wvT[:],
                        start=True, stop=True,
                    )
                nb = len(blocks)
                vdst = v_sb[:, p, :, :].rearrange(
                    "p c (h x) -> p c h x", x=VPAD)[:, 0:nb, :, 0:HD]
                src = ps[:, 0:nb * E].rearrange(
                    "p (c h x) -> p c h x", c=nb, x=HD)
                if scalar_eng:
                    nc.scalar.activation(
                        vdst, src, mybir.ActivationFunctionType.Copy)
                else:
                    nc.vector.tensor_copy(vdst, src)

            def wo_proj(nt, ebs=(0, 1)):
                # mixT[e'] = sum_h WoT_h.T @ ctxn_h  (+ bop, on DVE)
                for eb in ebs:
                    ps = ps_big.tile([P, 1024], F32, tag="big")
                    for h in range(H):
                        nc.tensor.matmul(
                            ps[:, 0:512],
                            woT[:, h, eb * P:(eb + 1) * P],
                            ctxn[:, h, nt * 512:(nt + 1) * 512],
                            start=(h == 0), stop=(h == H - 1),
                        )
                    nc.vector.tensor_scalar_add(
                        mixT[:, eb, nt * 512:(nt + 1) * 512], ps[:, 0:512],
                        bop[:, eb:eb + 1])
                    nc.sync.dma_start(
                        mixT_d[eb][:, nt * 512:(nt + 1) * 512],
                        mixT[:, eb, nt * 512:(nt + 1) * 512])

            def exp_block(j, s, dst):
                # dst: fp8 [P, 1024] plane of the pair tile; scores arrive
                # pre-scaled by SLOPE (folded into kT)
                if j in act_blocks:
                    nc.scalar.activation(
                        dst, s[:],
                        mybir.ActivationFunctionType.Exp,
                        bias=maskb[:, j:j + 1], scale=0.125 / SLOPE)
                else:
                    nc.vector.tensor_scalar_add(
                        dst.bitcast(I8), s[:], CCONST)

            # k tiles 0-1 (pcT chunk 0), then q, then the remaining k
            # tiles; v is interleaved into the first attention pass
            k_proj_eb(0, 0, 1)
            q_proj_ebnt(0, 0)
            # prologue v (pairs 0-1) on ScalarE: the DVE's first exp isn't
            # queued behind them
            v_pair(0, scalar_eng=True)
            if npair > 1:
                v_pair(1, scalar_eng=True)
            ke0 = [("k", 0, nt0, min(2, knt - nt0))
                   for nt0 in range(2, knt, 2)]
            ke1 = [("k", 1, nt0, min(2, knt - nt0))
                   for nt0 in range(0, knt, 2)]
            extras_p0 = [("k", 0, 1, 1)] + ke0 + [ke1[0], ("q", 1, 0, 0)]
            extras_p1 = ke1[1:] + [("q", 0, 1, 0), ("q", 1, 1, 0)]

            # ---- attention main loop (software-pipelined over pairs) ----
            # At pass end the raw ctx+Z rows are staged PSUM->SBUF with one
            # ScalarE copy per head (instantly freeing the accumulators);
            # the reciprocal/broadcast/multiply tail is DEFERRED into the
            # next pass so pass transitions never serialize the exp
            # engines / idle the PE (which would re-throttle HAM).
            def make_norm(h, stagec, zrow, nt):
                def phase1():
                    zr = spool.tile([1, 512], F32, tag="zr")
                    nc.vector.reciprocal_approx_fast(zr[:], zrow[:])
                    return zr

                def phase2(zr):
                    # broadcast AND multiply on gpsimd -- all operands are
                    # SBUF here (stagec/zbs/ctxn), so it's legal, and it
                    # keeps the DVE queue clear for exps at pass starts
                    zbs = spool.tile([HD, 512], F32, tag="zbs")
                    nc.gpsimd.partition_broadcast(zbs[:], zr[:])
                    nc.gpsimd.tensor_mul(
                        ctxn[:, h, nt * 512:(nt + 1) * 512],
                        stagec[:], zbs[:])

                state = {}
                return [lambda: state.__setitem__('zr', phase1()),
                        lambda: phase2(state['zr'])]

            pending_norm = []
            carry = None
            passes = [(0, 0), (1, 0), (0, 1), (1, 1)]
            for pi, (hp, nt) in enumerate(passes):
                h0, h1 = 2 * hp, 2 * hp + 1
                acc0 = ps_ctx.tile([HD + 1, 512], F32, tag="ctx")
                acc1 = ps_ctx.tile([HD + 1, 512], F32, tag="ctx")
                pend = None

                def ctx_pair(a, p, single, last):
                    for acc, h in ((acc0, h0), (acc1, h1)):
                        vsl = v_sb[:, p, :, h * VPAD:h * VPAD + HD + 1]
                        asl = a[:, :, (h % 2) * 512:(h % 2 + 1) * 512]
                        if single:
                            nc.tensor.matmul(
                                acc[:], vsl[:, 0, :], asl[:, 0, :],
                                start=(p == 0), stop=last)
                        else:
                            nc.tensor.matmul(
                                acc[:], vsl, asl,
                                start=(p == 0), stop=last,
                                perf_mode=mybir.MatmulPerfMode.DoubleRow)

                for p in range(npair):
                    single = odd and p == npair - 1
                    js = [2 * p] if single else [2 * p, 2 * p + 1]
                    a = attn_pool.tile([P, 2, 1024], F8, tag="attn")
                    for idx, j in enumerate(js):
                        s = ps_big.tile([P, 1024], F32, tag="big")
                        # scores for the two heads -> adjacent psum banks;
                        # 64-row lhsT slices on disjoint row groups run
                        # concurrently on the PE
                        nc.tensor.matmul(
                            s[:, 0:512],
                            kT[0:HD, hp, j * P:(j + 1) * P],
                            qT[0:HD, hp, nt * 512:(nt + 1) * 512],
                            start=True, stop=True,
                        )
                        nc.tensor.matmul(
                            s[:, 512:1024],
                            kT[HD:P, hp, j * P:(j + 1) * P],
                            qT[HD:P, hp, nt * 512:(nt + 1) * 512],
                            start=True, stop=True,
                        )
                        exp_block(j, s, a[:, idx, :])
                        if pi == 0 and idx == 0 and p + 2 < npair:
                            # v pair 2 ahead of ctx; alternate engines
                            v_pair(p + 2, scalar_eng=p % 2 == 0)
                        extras = (extras_p0 if pi == 0 else
                                  extras_p1 if pi == 1 else [])
                        if j % 2 == 1 and (j - 1) // 2 < len(extras):
                            kind, eb, nt0, n_nt = extras[(j - 1) // 2]
                            if kind == "k":
                                k_proj_eb(eb, nt0, n_nt)
                            else:
                                q_proj_ebnt(eb, nt0)
                        if pi == 2 and j == 10:
                            wo_proj(0, ebs=(0,))
                        if pi == 3 and j == 10:
                            wo_proj(0, ebs=(1,))
                    # zero-contribution matmul (adds 0*0 into the live
                    # accumulator) keeps the PE activity monitor from
                    # re-throttling the clock during exp-wait gaps
                    if p > 0:
                        nc.tensor.matmul(acc0[:, 0:256], warm[0:P, 0:HD + 1],
                                         warm[:, 0:256],
                                         start=False, stop=False,
                                         skip_group_check=True)
                    if p == 0 and carry is not None:
                        # previous pass's final ctx + staging, emitted
                        # AFTER this pass's first scores/exps so neither
                        # the PE FIFO nor the ACT queue stalls the new
                        # pass at the transition
                        carry()
                        carry = None
                    elif pending_norm:
                        pending_norm.pop(0)()
                    if pend is not None:
                        ctx_pair(*pend, last=False)
                    pend = (a, p, single)

                def make_carry(ctx_pair, pend, acc0, acc1, h0, h1, nt):
                    def run():
                        nonlocal pending_norm
                        ctx_pair(*pend, last=True)
                        # stage ctx rows + Z row to SBUF on ScalarE (the
                        # Z copy is the one legal cross-partition hop:
                        # 64 -> 0), freeing both accumulators
                        pending_norm = []
                        for h, acc in ((h0, acc0), (h1, acc1)):
                            stagec = spool.tile([HD, 512], F32, tag="stage")
                            zrow = spool.tile([1, 512], F32, tag="zrow")
                            nc.scalar.activation(
                                stagec[:], acc[0:HD, :],
                                mybir.ActivationFunctionType.Copy)
                            nc.scalar.activation(
                                zrow[:], acc[HD:HD + 1, :],
                                mybir.ActivationFunctionType.Copy)
                            pending_norm += make_norm(h, stagec, zrow, nt)
                    return run

                carry = make_carry(ctx_pair, pend, acc0, acc1, h0, h1, nt)
                if pi == len(passes) - 1:
                    carry()
                    for f in pending_norm:
                        f()
                    pending_norm = []
            if jmax <= 10:
                wo_proj(0)
            wo_proj(1)

    nc.finalize()
    return nc


def _get_nc(jmax, jfull):
    key = (jmax, jfull)
    if key not in _CACHE:
        _CACHE[key] = build_nc(jmax, jfull)
    return _CACHE[key]


def kernel(mesh_feats, pc_feats, Wq, Wk, Wv, bq, bk, bv, Wo, bo, lengths,
           _trace=False, _trace_kwargs=None):
    mesh_feats = np.asarray(mesh_feats, np.float32)
    pc_feats = np.asarray(pc_feats, np.float32)
    Wq, Wk, Wv = (np.asarray(x, np.float32) for x in (Wq, Wk, Wv))
    bqv, bkv, bvv = (np.asarray(x, np.float32) for x in (bq, bk, bv))
    Wo, bo = np.asarray(Wo, np.float32), np.asarray(bo, np.float32)
    lengths = np.asarray(lengths, np.int32)

    bf = ml_dtypes.bfloat16
    f8 = ml_dtypes.float8_e4m3
    wqT = np.ascontiguousarray(
        Wq.T.reshape(2, P, E).transpose(1, 0, 2)).astype(bf)   # [128, 2, 256]
    wkT = np.ascontiguousarray(Wk.T).astype(bf)          # [128, 256]
    wvT = np.ascontiguousarray(Wv.T).astype(bf)          # [128, 256]
    woT = np.ascontiguousarray(
        Wo.T.reshape(H, HD, E).transpose(1, 0, 2)).astype(bf)  # [64, 4, 256]
    bq2 = np.ascontiguousarray(bqv.reshape(2, P).T)      # [128, 2]
    bk2 = np.ascontiguousarray(bkv.reshape(2, P).T)
    bop = Wo @ bvv + bo
    bop2 = np.ascontiguousarray(bop.reshape(2, P).T)

    jmax = int(min(NKB, max(1, math.ceil(int(lengths.max()) / 128))))
    jfull = int(min(NKB, int(lengths.min()) // 128))

    idx = np.arange(NK).reshape(NKB, P).T                # [128, 32]
    in_maps = []
    for c in range(8):
        b, half = c // 2, c % 2
        meshT = np.ascontiguousarray(
            mesh_feats[b, half * NQH:(half + 1) * NQH, :].T
            .reshape(2, P, NQH).transpose(1, 0, 2)).astype(f8)  # [128,2,1024]
        pcT = np.ascontiguousarray(pc_feats[b].T).astype(f8)
        maskb = np.where(idx < int(lengths[b]), 0.0, MASK_NEG).astype(np.float32)
        consts = np.ascontiguousarray(
            np.concatenate([bk2, bq2, bop2, maskb], axis=1).astype(np.float32))
        in_maps.append({
            "meshT": meshT, "pcT": pcT, "wqT": wqT, "wkT": wkT,
            "wvT": wvT, "woT": woT, "consts": consts,
        })

    nc = _get_nc(jmax, jfull)
    res = run_bass_kernel_spmd(
        nc, in_maps, list(range(8)),
        trace=_trace, **(_trace_kwargs or {}))
    out = np.empty((B, NQ, 2 * E), np.float32)
    out[:, :, :E] = mesh_feats
    for c in range(8):
        b, half = c // 2, c % 2
        mixT = res.results[c]["mixT"]            # [2, 128, NQH] bf16
        out[b, half * NQH:(half + 1) * NQH, E:] = \
            mixT.astype(np.float32).reshape(E, NQH).T
    if _trace:
        return out, res
    return out


# revision 47
# speedup vs baseline: 1.7084x; 1.7084x over previous
"""AttentionMixer kernel for 8 Trainium2 NeuronCores.

Sharding: data-parallel over (batch B=4) x (query-half NQ/2) -> 8 cores.
Each core computes, for its (b, half):
    q = meshT slice proj, k/v = pc proj (k/v work duplicated across the
    2 cores of a batch), masked softmax attention, Wo projection.
Layout is "transposed" throughout (features on partitions, tokens on the
free dim) so every matmul contracts over the partition dim natively:
    qT/kT: [e, n] via W.T as lhsT, xT as rhs (bf16)
    scoresT: [nk, nq] = kT_h.T-contract-d qT_h  (2 heads row-packed)
    attnT = exp(scoresT/8 + mask) in fp8e4m3, produced ALTERNATELY by
      - ScalarE: native Exp ACTIVATE (handles masked/partial blocks)
      - VectorE: Schraudolph bit-trick -- one tensor_scalar op computes
        int8(round(1.4427*s + 55.63)) whose int8 bits ARE the e4m3
        encoding of exp(s/8) to ~7% per-element (averages out over the
        ~2500-key softmax; tolerance is 2e-2).  Only fully-valid blocks
        (j < jfull on every core) go to VectorE so no mask is needed.
    ctxT_h: [65, nq] via fp8 DoubleRow matmuls -- one matmul contracts
      TWO 128-key blocks (virtual K=256), with a ones column appended to
      v giving the softmax denominator Z for free.  mix = ctx/Z @ Wo.T.
Projections/scores bf16 with fp32 PSUM accumulation; inputs pcT/meshT
are fp8 (error averages out through the 128-dim projections).

The j-loop is software-pipelined at block-pair granularity: scores+exp
of pair p are emitted before ctx of pair p-1 so PE never stalls behind
the current exp, and the two exp engines run concurrently.
"""

import math

import numpy as np
import ml_dtypes

import concourse.bass as bass
import concourse.bacc as bacc
import concourse.mybir as mybir
import concourse.tile as tile
from concourse.bass_utils import run_bass_kernel_spmd

B, NQ, NK, E, DPC, H = 4, 2048, 4096, 256, 128, 4
HD = E // H  # 64
NQH = NQ // 2  # per-core queries: 1024
NKB = NK // 128  # 32 nk blocks
P = 128
BF16 = mybir.dt.bfloat16
F32 = mybir.dt.float32
F8 = mybir.dt.float8e4
I8 = mybir.dt.int8
MASK_NEG = -80.0
# Schraudolph exp in e4m3-bit space: bits = round(SLOPE*s + CCONST)
# (DVE fp32->int8 convert is round-half-even, probed on HW).
SLOPE = 8 * math.log2(math.e) * 0.125   # 1.4427
CCONST = 55.63
VPAD = 80  # per-head stride in v_sb (65 used; *H must be 16B aligned)

_CACHE = {}


def build_nc(jmax=NKB, jfull=NKB):
    nc = bacc.Bacc(None)
    knt = (jmax + 3) // 4         # 512-wide kT tiles needed
    nch = (jmax + 7) // 8         # 1024-wide pcT DMA chunks needed
    npair = (jmax + 1) // 2       # ctx block-pairs (last may be single)
    odd = jmax % 2 == 1

    # blocks that must use ScalarE (need the mask bias); plus alternating
    # leading blocks so the two exp engines get ~equal work
    tail = list(range(jfull, jmax))
    n_act_extra = max(0, (jmax + 1) // 2 - len(tail))
    act_blocks = set(tail) | set(range(jfull)[1::2][:n_act_extra])

    # ---- DRAM params (per-core shapes; host stages exact SBUF layouts) ----
    meshT_d = nc.declare_dram_parameter("meshT", [P, 2, NQH], F8, False)
    pcT_d = nc.declare_dram_parameter("pcT", [P, NK], F8, False)
    wqT_d = nc.declare_dram_parameter("wqT", [P, 2, E], BF16, False)
    wkT_d = nc.declare_dram_parameter("wkT", [P, E], BF16, False)
    wvT_d = nc.declare_dram_parameter("wvT", [P, E], BF16, False)
    woT_d = nc.declare_dram_parameter("woT", [HD, H, E], BF16, False)
    # consts: [bk | bq | bop | maskb] along the free dim
    consts_d = nc.declare_dram_parameter("consts", [P, 6 + NKB], F32, False)
    mixT_d = nc.declare_dram_parameter("mixT", [2, P, NQH], BF16, isOutput=True)

    with tile.TileContext(nc) as tc:
        with (
            tc.tile_pool(name="const", bufs=1) as cpool,
            tc.tile_pool(name="acts", bufs=1) as apool,
            tc.tile_pool(name="attn", bufs=4) as attn_pool,
            tc.tile_pool(name="small", bufs=2) as spool,
            tc.tile_pool(name="ps_big", bufs=3, space="PSUM") as ps_big,
            tc.tile_pool(name="ps_ctx", bufs=2, space="PSUM") as ps_ctx,
        ):
            # ---- load constants / inputs into SBUF ----
            meshT = cpool.tile([P, 2, NQH], F8)
            pcT = cpool.tile([P, NK], F8)
            wqT = cpool.tile([P, 2, E], BF16)
            wkT = cpool.tile([P, E], BF16)
            wvT = cpool.tile([P, E], BF16)
            woT = cpool.tile([HD, H, E], BF16)
            consts = cpool.tile([P, 6 + NKB], F32)
            # consts[:, 0:2] holds bk, unused: the k-bias adds a per-query
            # constant to every key's score, which softmax cancels.
            bq = consts[:, 2:4]
            bop = consts[:, 4:6]
            maskb = consts[:, 6:6 + NKB]

            # critical-path inputs on the sync engine (HWDGE)
            nc.sync.dma_start(wkT[:], wkT_d[:, :])
            nc.sync.dma_start(consts[:], consts_d[:, :])
            nc.sync.dma_start(pcT[:, 0:512], pcT_d[:, 0:512])
            nc.sync.dma_start(wqT[:], wqT_d[:, :, :])
            nc.sync.dma_start(meshT[:, 0, :], meshT_d[:, 0, :])
            nc.sync.dma_start(meshT[:, 1, :], meshT_d[:, 1, :])
            nc.sync.dma_start(pcT[:, 512:1024], pcT_d[:, 512:1024])
            # bulk inputs ride the gpsimd SWDGE queue so they don't
            # serialize behind the critical-path loads above
            nc.gpsimd.dma_start(wvT[:], wvT_d[:, :])
            for ch in range(1, nch):
                nc.gpsimd.dma_start(pcT[:, ch * 1024:(ch + 1) * 1024],
                                    pcT_d[:, ch * 1024:(ch + 1) * 1024])
            nc.gpsimd.dma_start(woT[:], woT_d[:, :, :])

            # HAM warm-up: dependency-free matmuls during the input-DMA
            # window so the PE clock gate is at 2.4 GHz for the real work
            warm = cpool.tile([P, 512], BF16)
            nc.vector.memset(warm[:], 0.0)
            wps = ps_big.tile([P, 1024], F32, tag="big")
            for _ in range(20):
                nc.tensor.matmul(wps[:, 0:512], warm[:, 0:128], warm[:],
                                 start=True, stop=True)

            kT = apool.tile([P, 2, knt * 512], BF16)
            qT = apool.tile([P, 2, NQH], BF16)
            # v (fp8) in DoubleRow pair layout + ones column for Z
            v_sb = apool.tile([P, npair, 2, H * VPAD], F8)
            for h in range(H):
                nc.vector.memset(
                    v_sb[:, :, :, h * VPAD + HD:h * VPAD + HD + 1], 1.0)
            mixT = apool.tile([P, 2, NQH], BF16)
            ctxn = apool.tile([HD, H, NQH], BF16)  # normalized ctxT per head

            def k_proj_eb(eb, nt0, n_nt):
                # n_nt (1 or 2) 512-wide kT tiles for one e-block; the
                # PSUM->SBUF casts are split across ScalarE/VectorE so
                # neither exp engine eats the whole cost
                ps = ps_big.tile([P, 1024], F32, tag="big")
                for i in range(n_nt):
                    nc.tensor.matmul(
                        ps[:, i * 512:(i + 1) * 512],
                        wkT[:, eb * P:(eb + 1) * P],
                        pcT[:, (nt0 + i) * 512:(nt0 + i + 1) * 512],
                        start=True, stop=True,
                    )
                    # kT is pre-scaled by SLOPE so the DVE exp needs only a
                    # single ADD op and ACT exp just rescales
                    dst = kT[:, eb, (nt0 + i) * 512:(nt0 + i + 1) * 512]
                    if i == 0:
                        nc.scalar.activation(
                            dst, ps[:, 0:512],
                            mybir.ActivationFunctionType.Copy, scale=SLOPE)
                    else:
                        nc.vector.tensor_scalar_mul(
                            dst, ps[:, 512:1024], SLOPE)

            def q_proj_ebnt(eb, nt):
                ps = ps_big.tile([P, 1024], F32, tag="big")
                for cb in range(2):
                    nc.tensor.matmul(
                        ps[:, 0:512],
                        wqT[:, cb, eb * P:(eb + 1) * P],
                        meshT[:, cb, nt * 512:(nt + 1) * 512],
                        start=(cb == 0), stop=(cb == 1),
                    )
                nc.vector.tensor_scalar_add(
                    qT[:, eb, nt * 512:(nt + 1) * 512], ps[:, 0:512],
                    bq[:, eb:eb + 1])

            def v_pair(p, scalar_eng=False):
                # both blocks of a pair -> one ps tile, one engine copy
                # (gpsimd can't help: it has no PSUM access)
                blocks = [2 * p] if odd and p == npair - 1 else [2 * p, 2 * p + 1]
                ps = ps_big.tile([P, 1024], F32, tag="big")
                for ci, j in enumerate(blocks):
                    nc.tensor.matmul(
                        ps[:, ci * E:(ci + 1) * E],
                        pcT[:, j * P:(j + 1) * P],
                        wvT[:],
                        start=True, stop=True,
                    )
                nb = len(blocks)
                vdst = v_sb[:, p, :, :].rearrange(
                    "p c (h x) -> p c h x", x=VPAD)[:, 0:nb, :, 0:HD]
                src = ps[:, 0:nb * E].rearrange(
                    "p (c h x) -> p c h x", c=nb, x=HD)
                if scalar_eng:
                    nc.scalar.activation(
                        vdst, src, mybir.ActivationFunctionType.Copy)
                else:
                    nc.vector.tensor_copy(vdst, src)

            def wo_proj(nt, ebs=(0, 1)):
                # mixT[e'] = sum_h WoT_h.T @ ctxn_h  (+ bop, on DVE)
                for eb in ebs:
                    ps = ps_big.tile([P, 1024], F32, tag="big")
                    for h in range(H):
                        nc.tensor.matmul(
                            ps[:, 0:512],
                            woT[:, h, eb * P:(eb + 1) * P],
                            ctxn[:, h, nt * 512:(nt + 1) * 512],
                            start=(h == 0), stop=(h == H - 1),
                        )
                    nc.vector.tensor_scalar_add(
                        mixT[:, eb, nt * 512:(nt + 1) * 512], ps[:, 0:512],
                        bop[:, eb:eb + 1])
                    nc.sync.dma_start(
                        mixT_d[eb][:, nt * 512:(nt + 1) * 512],
                        mixT[:, eb, nt * 512:(nt + 1) * 512])

            def exp_block(j, s, dst):
                # dst: fp8 [P, 1024] plane of the pair tile; scores arrive
                # pre-scaled by SLOPE (folded into kT)
                if j in act_blocks:
                    nc.scalar.activation(
                        dst, s[:],
                        mybir.ActivationFunctionType.Exp,
                        bias=maskb[:, j:j + 1], scale=0.125 / SLOPE)
                else:
                    nc.vector.tensor_scalar_add(
                        dst.bitcast(I8), s[:], CCONST)

            # k tiles 0-1 (pcT chunk 0), then q, then the remaining k
            # tiles; v is interleaved into the first attention pass
            k_proj_eb(0, 0, 1)
            q_proj_ebnt(0, 0)
            # prologue v (pairs 0-1) on ScalarE: the DVE's first exp isn't
            # queued behind them
            v_pair(0, scalar_eng=True)
            if npair > 1:
                v_pair(1, scalar_eng=True)
            ke0 = [("k", 0, nt0, min(2, knt - nt0))
                   for nt0 in range(2, knt, 2)]
            ke1 = [("k", 1, nt0, min(2, knt - nt0))
                   for nt0 in range(0, knt, 2)]
            extras_p0 = [("k", 0, 1, 1)] + ke0 + [ke1[0], ("q", 1, 0, 0)]
            extras_p1 = ke1[1:] + [("q", 0, 1, 0), ("q", 1, 1, 0)]

            # ---- attention main loop (software-pipelined over pairs) ----
            # At pass end the raw ctx+Z rows are staged PSUM->SBUF with one
            # ScalarE copy per head (instantly freeing the accumulators);
            # the reciprocal/broadcast/multiply tail is DEFERRED into the
            # next pass so pass transitions never serialize the exp
            # engines / idle the PE (which would re-throttle HAM).
            def make_norm(h, stagec, zrow, nt):
                def phase1():
                    zr = spool.tile([1, 512], F32, tag="zr")
                    nc.vector.reciprocal_approx_fast(zr[:], zrow[:])
                    return zr

                def phase2(zr):
                    zbs = spool.tile([HD, 512], F32, tag="zbs")
                    nc.gpsimd.partition_broadcast(zbs[:], zr[:])
                    nc.vector.tensor_mul(
                        ctxn[:, h, nt * 512:(nt + 1) * 512],
                        stagec[:], zbs[:])

                state = {}
                return [lambda: state.__setitem__('zr', phase1()),
                        lambda: phase2(state['zr'])]

            pending_norm = []
            carry = None
            passes = [(0, 0), (1, 0), (0, 1), (1, 1)]
            for pi, (hp, nt) in enumerate(passes):
                h0, h1 = 2 * hp, 2 * hp + 1
                acc0 = ps_ctx.tile([HD + 1, 512], F32, tag="ctx")
                acc1 = ps_ctx.tile([HD + 1, 512], F32, tag="ctx")
                pend = None

                def ctx_pair(a, p, single, last):
                    for acc, h in ((acc0, h0), (acc1, h1)):
                        vsl = v_sb[:, p, :, h * VPAD:h * VPAD + HD + 1]
                        asl = a[:, :, (h % 2) * 512:(h % 2 + 1) * 512]
                        if single:
                            nc.tensor.matmul(
                                acc[:], vsl[:, 0, :], asl[:, 0, :],
                                start=(p == 0), stop=last)
                        else:
                            nc.tensor.matmul(
                                acc[:], vsl, asl,
                                start=(p == 0), stop=last,
                                perf_mode=mybir.MatmulPerfMode.DoubleRow)

                for p in range(npair):
                    single = odd and p == npair - 1
                    js = [2 * p] if single else [2 * p, 2 * p + 1]
                    a = attn_pool.tile([P, 2, 1024], F8, tag="attn")
                    for idx, j in enumerate(js):
                        s = ps_big.tile([P, 1024], F32, tag="big")
                        # scores for the two heads -> adjacent psum banks;
                        # 64-row lhsT slices on disjoint row groups run
                        # concurrently on the PE
                        nc.tensor.matmul(
                            s[:, 0:512],
                            kT[0:HD, hp, j * P:(j + 1) * P],
                            qT[0:HD, hp, nt * 512:(nt + 1) * 512],
                            start=True, stop=True,
                        )
                        nc.tensor.matmul(
                            s[:, 512:1024],
                            kT[HD:P, hp, j * P:(j + 1) * P],
                            qT[HD:P, hp, nt * 512:(nt + 1) * 512],
                            start=True, stop=True,
                        )
                        exp_block(j, s, a[:, idx, :])
                        if pi == 0 and idx == 0 and p + 2 < npair:
                            # v pair 2 ahead of ctx; alternate engines
                            v_pair(p + 2, scalar_eng=p % 2 == 0)
                        extras = (extras_p0 if pi == 0 else
                                  extras_p1 if pi == 1 else [])
                        if j % 2 == 1 and (j - 1) // 2 < len(extras):
                            kind, eb, nt0, n_nt = extras[(j - 1) // 2]
                            if kind == "k":
                                k_proj_eb(eb, nt0, n_nt)
                            else:
                                q_proj_ebnt(eb, nt0)
                        if pi == 2 and j == 10:
                            wo_proj(0, ebs=(0,))
                        if pi == 3 and j == 10:
                            wo_proj(0, ebs=(1,))
                    # zero-contribution matmul (adds 0*0 into the live
                    # accumulator) keeps the PE activity monitor from
                    # re-throttling the clock during exp-wait gaps
                    if p > 0:
                        nc.tensor.matmul(acc0[:, 0:256], warm[0:P, 0:HD + 1],
                                         warm[:, 0:256],
                                         start=False, stop=False,
                                         skip_group_check=True)
                    if p == 0 and carry is not None:
                        # previous pass's final ctx + staging, emitted
                        # AFTER this pass's first scores/exps so neither
                        # the PE FIFO nor the ACT queue stalls the new
                        # pass at the transition
                        carry()
                        carry = None
                    elif pending_norm:
                        pending_norm.pop(0)()
                    if pend is not None:
                        ctx_pair(*pend, last=False)
                    pend = (a, p, single)

                def make_carry(ctx_pair, pend, acc0, acc1, h0, h1, nt):
                    def run():
                        nonlocal pending_norm
                        ctx_pair(*pend, last=True)
                        # stage ctx rows + Z row to SBUF on ScalarE (the
                        # Z copy is the one legal cross-partition hop:
                        # 64 -> 0), freeing both accumulators
                        pending_norm = []
                        for h, acc in ((h0, acc0), (h1, acc1)):
                            stagec = spool.tile([HD, 512], F32, tag="stage")
                            zrow = spool.tile([1, 512], F32, tag="zrow")
                            nc.scalar.activation(
                                stagec[:], acc[0:HD, :],
                                mybir.ActivationFunctionType.Copy)
                            nc.scalar.activation(
                                zrow[:], acc[HD:HD + 1, :],
                                mybir.ActivationFunctionType.Copy)
                            pending_norm += make_norm(h, stagec, zrow, nt)
                    return run

                carry = make_carry(ctx_pair, pend, acc0, acc1, h0, h1, nt)
                if pi == len(passes) - 1:
                    carry()
                    for f in pending_norm:
                        f()
                    pending_norm = []
            if jmax <= 10:
                wo_proj(0)
            wo_proj(1)

    nc.finalize()
    return nc


def _get_nc(jmax, jfull):
    key = (jmax, jfull)
    if key not in _CACHE:
        _CACHE[key] = build_nc(jmax, jfull)
    return _CACHE[key]


def kernel(mesh_feats, pc_feats, Wq, Wk, Wv, bq, bk, bv, Wo, bo, lengths,
           _trace=False, _trace_kwargs=None):
    mesh_feats = np.asarray(mesh_feats, np.float32)
    pc_feats = np.asarray(pc_feats, np.float32)
    Wq, Wk, Wv = (np.asarray(x, np.float32) for x in (Wq, Wk, Wv))
    bqv, bkv, bvv = (np.asarray(x, np.float32) for x in (bq, bk, bv))
    Wo, bo = np.asarray(Wo, np.float32), np.asarray(bo, np.float32)
    lengths = np.asarray(lengths, np.int32)

    bf = ml_dtypes.bfloat16
    f8 = ml_dtypes.float8_e4m3
    wqT = np.ascontiguousarray(
        Wq.T.reshape(2, P, E).transpose(1, 0, 2)).astype(bf)   # [128, 2, 256]
    wkT = np.ascontiguousarray(Wk.T).astype(bf)          # [128, 256]
    wvT = np.ascontiguousarray(Wv.T).astype(bf)          # [128, 256]
    woT = np.ascontiguousarray(
        Wo.T.reshape(H, HD, E).transpose(1, 0, 2)).astype(bf)  # [64, 4, 256]
    bq2 = np.ascontiguousarray(bqv.reshape(2, P).T)      # [128, 2]
    bk2 = np.ascontiguousarray(bkv.reshape(2, P).T)
    bop = Wo @ bvv + bo
    bop2 = np.ascontiguousarray(bop.reshape(2, P).T)

    jmax = int(min(NKB, max(1, math.ceil(int(lengths.max()) / 128))))
    jfull = int(min(NKB, int(lengths.min()) // 128))

    idx = np.arange(NK).reshape(NKB, P).T                # [128, 32]
    in_maps = []
    for c in range(8):
        b, half = c // 2, c % 2
        meshT = np.ascontiguousarray(
            mesh_feats[b, half * NQH:(half + 1) * NQH, :].T
            .reshape(2, P, NQH).transpose(1, 0, 2)).astype(f8)  # [128,2,1024]
        pcT = np.ascontiguousarray(pc_feats[b].T).astype(f8)
        maskb = np.where(idx < int(lengths[b]), 0.0, MASK_NEG).astype(np.float32)
        consts = np.ascontiguousarray(
            np.concatenate([bk2, bq2, bop2, maskb], axis=1).astype(np.float32))
        in_maps.append({
            "meshT": meshT, "pcT": pcT, "wqT": wqT, "wkT": wkT,
            "wvT": wvT, "woT": woT, "consts": consts,
        })

    nc = _get_nc(jmax, jfull)
    res = run_bass_kernel_spmd(
        nc, in_maps, list(range(8)),
        trace=_trace, **(_trace_kwargs or {}))
    out = np.empty((B, NQ, 2 * E), np.float32)
    out[:, :, :E] = mesh_feats
    for c in range(8):
        b, half = c // 2, c % 2
        mixT = res.results[c]["mixT"]            # [2, 128, NQH] bf16
        out[b, half * NQH:(half + 1) * NQH, E:] = \
            mixT.astype(np.float32).reshape(E, NQH).T
    if _trace:
        return out, res
    return out
